# revision 8
# baseline (speedup 1.0000x reference)
"""Multi-head attention (B=2, S=2048, H=1024, 16 heads) on 8 TRN2 NeuronCores.

Sharding (tensor-parallel heads x data-parallel batch, per the hint):
  core c -> batch b = c // 4, head group g = c % 4 (4 heads each).
The 4 partial outputs per batch are summed on the host during unsharding
(Megatron-style TP partial sums); bv/bo fold into a host-side additive
constant (bv @ Wo + bo), which is exact.

v3 vs the original baseline:
  * q-block-major attention (j outer, head inner) so the out-projection and
    output DMA overlap later blocks' attention instead of serializing in a
    ~50us tail.
  * The Scalar (activation) engine runs EXP exclusively -- it is the
    critical engine at ~129us (16.8M probs per core at ~1 elem/cycle); all
    psum casts/copies are on DVE, broadcasts on GPSIMD, moves on DMA.
  * Optional CTX_SPLIT: the probs@V contraction is split into two K=64
    row-group tiles (partitions 0:64 / 64:128) aimed at concurrent PE
    row-tile execution, with the two partial accumulations summed on DVE.
  * fp16 scores path, bf16 probs/V/ctx path (identical numerics to the
    baseline, rel err ~3.5e-3; fp8 was tried and rejected: softmax
    concentration exposes the ~4% elementwise quantization error nearly
    unattenuated in max-error terms).
"""

import ml_dtypes
import numpy as np

import concourse.bacc as bacc
import concourse.mybir as mybir
import concourse.tile as tile
from concourse.bass_utils import run_bass_kernel_spmd

NCORES = 8
B, S, HID = 2, 2048, 1024
NH, HD = 16, 64
HPC = 4            # heads per core
QC = HPC * HD      # 256 local projection cols per core
HC = HID // 128    # 8 hidden chunks
TC = S // 128      # 16 token chunks
NJ = S // 512      # 4 q blocks

F32 = mybir.dt.float32
BF16 = mybir.dt.bfloat16
FP16 = mybir.dt.float16
EXP = mybir.ActivationFunctionType.Exp
MULT = mybir.AluOpType.mult
ADD = mybir.AluOpType.add

# Split probs@V into two K=64 row-group tiles: measured no-gain (the PE
# streams N rows per matmul regardless of K; pairing only helps matmuls
# that are K<=64 to begin with, i.e. the scores).
CTX_SPLIT = False


def build_nc():
    nc = bacc.Bacc("TRN2", target_bir_lowering=False, debug=False,
                   num_devices=NCORES)
    xT = nc.declare_dram_parameter("xT", [HID, S], FP16, isOutput=False)
    wq = nc.declare_dram_parameter("wq", [HID, QC], FP16, isOutput=False)
    wk = nc.declare_dram_parameter("wk", [HID, QC], FP16, isOutput=False)
    wv = nc.declare_dram_parameter("wv", [HID, QC], FP16, isOutput=False)
    wo = nc.declare_dram_parameter("wo", [QC, HID], BF16, isOutput=False)
    bq = nc.declare_dram_parameter("bq", [QC], F32, isOutput=False)
    bk = nc.declare_dram_parameter("bk", [QC], F32, isOutput=False)
    ident = nc.declare_dram_parameter("ident", [128, 128], BF16, isOutput=False)
    out = nc.declare_dram_parameter("out", [S, HID], BF16, isOutput=True)

    with tile.TileContext(nc) as tc:
        with (
            tc.tile_pool(name="const", bufs=1) as constp,
            tc.tile_pool(name="qkv", bufs=1) as qkvp,
        ):
            idb_sb = constp.tile([128, 128], BF16)
            bq_sb = constp.tile([128, 2], F32)
            bk_sb = constp.tile([128, 2], F32)
            wo_sb = constp.tile([128, 2 * HID], BF16)

            # Q^T/K^T per head (fp16), duplicated across both partition
            # halves so chunk-pair scores matmuls alternate halves.
            qt2 = qkvp.tile([128, HPC * S], FP16)
            kt2 = qkvp.tile([128, HPC * S], FP16)
            vt_sb = qkvp.tile([128, 2 * S], BF16)
            # Natural-layout V (bf16) in per-(tok-chunk, head) strips of 128
            # cols: 64 V cols then a 1.0 column for the softmax denominator.
            vnat = qkvp.tile([128, TC * HPC * 128], BF16)
            ctxf = qkvp.tile([128, 2 * S], BF16)

            nc.vector.memset(
                vnat[:, :].rearrange("p (s e) -> p s e", e=128)[:, :, HD:HD + 1],
                1.0)

            # ---- phase 1: projections (fp16), K -> Q -> V -----------------
            with (
                tc.tile_pool(name="xw", bufs=1) as xwp,
                tc.tile_pool(name="ps1", bufs=4, space="PSUM") as ps1,
            ):
                xT_sb = xwp.tile([128, HC * S], FP16)
                wq_sb = xwp.tile([128, HC * QC], FP16)
                wk_sb = xwp.tile([128, HC * QC], FP16)
                wv_sb = xwp.tile([128, HC * QC], FP16)

                for hc in range(HC):
                    r = slice(hc * 128, (hc + 1) * 128)
                    eng = nc.sync if hc % 2 == 0 else nc.gpsimd
                    eng.dma_start(xT_sb[:, hc * S:(hc + 1) * S], xT[r, :])
                    nc.scalar.dma_start(wk_sb[:, hc * QC:(hc + 1) * QC],
                                        wk[r, :])
                    nc.scalar.dma_start(wq_sb[:, hc * QC:(hc + 1) * QC],
                                        wq[r, :])
                    nc.scalar.dma_start(wv_sb[:, hc * QC:(hc + 1) * QC],
                                        wv[r, :])
                nc.scalar.dma_start(idb_sb[:, :], ident[:, :])
                for ci in range(2):
                    nc.sync.dma_start(bq_sb[:, ci:ci + 1],
                                      bq[ci * 128:(ci + 1) * 128])
                    nc.sync.dma_start(bk_sb[:, ci:ci + 1],
                                      bk[ci * 128:(ci + 1) * 128])
                    nc.scalar.dma_start(
                        wo_sb[:, ci * HID:(ci + 1) * HID],
                        wo[ci * 128:(ci + 1) * 128, :])

                def proj(w_sb, ci, jb, ps):
                    for hc in range(HC):
                        nc.tensor.matmul(
                            ps[:, :],
                            w_sb[:, hc * QC + ci * 128:hc * QC + ci * 128 + 128],
                            xT_sb[:, hc * S + jb * 512:hc * S + jb * 512 + 512],
                            start=(hc == 0), stop=(hc == HC - 1))

                for w_sb, dst, b_sb in ((wk_sb, kt2, bk_sb), (wq_sb, qt2, bq_sb)):
                    for ci in range(2):
                        hA, hB = 2 * ci, 2 * ci + 1
                        for jb in range(4):
                            ps = ps1.tile([128, 512], F32, tag="ps1")
                            proj(w_sb, ci, jb, ps)
                            o = jb * 512
                            nc.vector.tensor_scalar_add(
                                dst[0:64, hA * S + o:hA * S + o + 512],
                                ps[0:64, :], b_sb[0:64, ci:ci + 1])
                            nc.vector.tensor_scalar_add(
                                dst[64:128, hB * S + o:hB * S + o + 512],
                                ps[64:128, :], b_sb[64:128, ci:ci + 1])
                            nc.sync.dma_start(
                                dst[64:128, hA * S + o:hA * S + o + 512],
                                dst[0:64, hA * S + o:hA * S + o + 512])
                            nc.gpsimd.dma_start(
                                dst[0:64, hB * S + o:hB * S + o + 512],
                                dst[64:128, hB * S + o:hB * S + o + 512])
                for ci in range(2):
                    for jb in range(4):
                        ps = ps1.tile([128, 512], F32, tag="ps1")
                        proj(wv_sb, ci, jb, ps)
                        nc.vector.tensor_copy(
                            vt_sb[:, ci * S + jb * 512:ci * S + (jb + 1) * 512],
                            ps[:, :])

            # V^T -> natural V strips via PE transpose + DVE copy.
            with tc.tile_pool(name="trp", bufs=4, space="PSUM") as trp:
                for ci in range(2):
                    for t in range(TC):
                        tp = trp.tile([128, 128], BF16, tag="tr")
                        nc.tensor.transpose(
                            tp[:, :],
                            vt_sb[:, ci * S + t * 128:ci * S + t * 128 + 128],
                            idb_sb[:, :])
                        dst = vnat[:, (t * HPC + 2 * ci) * 128:
                                   (t * HPC + 2 * ci + 2) * 128].rearrange(
                            "p (h e) -> p h e", h=2)[:, :, 0:HD]
                        srcv = tp[:, :].rearrange(
                            "p (h e) -> p h e", h=2)[:, :, 0:HD]
                        nc.vector.tensor_copy(dst, srcv)

            # ---- phase 2: attention, q-block-major ------------------------
            with (
                tc.tile_pool(name="probs", bufs=4) as probsp,
                tc.tile_pool(name="div", bufs=2) as divp,
                tc.tile_pool(name="ostg", bufs=3) as ostg,
                tc.tile_pool(name="scps", bufs=2, space="PSUM") as scps,
                tc.tile_pool(name="ctps", bufs=1 if CTX_SPLIT else 2,
                             space="PSUM") as ctps,
                tc.tile_pool(name="ops", bufs=2, space="PSUM") as ops,
            ):
                for j in range(NJ):
                    for h in range(HPC):
                        hS = h * S
                        if CTX_SPLIT:
                            ctx_ps = ctps.tile([128, 1024], F32, tag="ctx")
                        else:
                            ctx_ps = ctps.tile([128, 512], F32, tag="ctx")
                        for cp in range(TC // 2):
                            c0, c1 = 2 * cp, 2 * cp + 1
                            sp = scps.tile([128, 1024], F32, tag="sc")
                            nc.tensor.matmul(
                                sp[:, 0:512],
                                kt2[0:64, hS + c0 * 128:hS + c0 * 128 + 128],
                                qt2[0:64, hS + j * 512:hS + j * 512 + 512],
                                start=True, stop=True, tile_position=(0, 0))
                            nc.tensor.matmul(
                                sp[:, 512:1024],
                                kt2[64:128, hS + c1 * 128:hS + c1 * 128 + 128],
                                qt2[64:128, hS + j * 512:hS + j * 512 + 512],
                                start=True, stop=True, tile_position=(64, 0))
                            probs = probsp.tile([128, 1024], BF16, tag="probs")
                            nc.scalar.activation(probs[:, :], sp[:, :], EXP)
                            for ck, coff in ((c0, 0), (c1, 512)):
                                strip = (ck * HPC + h) * 128
                                if CTX_SPLIT:
                                    nc.tensor.matmul(
                                        ctx_ps[0:HD + 1, 0:512],
                                        vnat[0:64, strip:strip + HD + 1],
                                        probs[0:64, coff:coff + 512],
                                        start=(cp == 0 and ck == c0),
                                        stop=(cp == TC // 2 - 1 and ck == c1),
                                        tile_position=(0, 0))
                                    nc.tensor.matmul(
                                        ctx_ps[0:HD + 1, 512:1024],
                                        vnat[64:128, strip:strip + HD + 1],
                                        probs[64:128, coff:coff + 512],
                                        start=(cp == 0 and ck == c0),
                                        stop=(cp == TC // 2 - 1 and ck == c1),
                                        tile_position=(64, 0))
                                else:
                                    nc.tensor.matmul(
                                        ctx_ps[0:HD + 1, :],
                                        vnat[:, strip:strip + HD + 1],
                                        probs[:, coff:coff + 512],
                                        start=(cp == 0 and ck == c0),
                                        stop=(cp == TC // 2 - 1 and ck == c1))
                        # divide by the denominator (row 64)
                        ci, lo = h // 2, (h % 2) * 64
                        craw = divp.tile([65, 512], F32, tag="craw")
                        if CTX_SPLIT:
                            nc.vector.tensor_tensor(
                                out=craw[0:65, :], in0=ctx_ps[0:65, 0:512],
                                in1=ctx_ps[0:65, 512:1024], op=ADD)
                        else:
                            nc.vector.tensor_copy(craw[0:65, :],
                                                  ctx_ps[0:65, :])
                        denr = divp.tile([128, 4], F32, tag="denr")
                        nc.sync.dma_start(denr[:, :], craw[64:65, :])
                        recr = divp.tile([128, 4], F32, tag="recr")
                        nc.vector.reciprocal(recr[:, :], denr[:, :])
                        rrow = divp.tile([1, 512], F32, tag="rrow")
                        nc.sync.dma_start(rrow[:, :], recr[:, :])
                        Dt = divp.tile([64, 512], F32, tag="Dt")
                        nc.gpsimd.partition_broadcast(Dt[:, :], rrow[0:1, :])
                        o = ci * S + j * 512
                        if lo == 0:
                            nc.vector.tensor_tensor(
                                out=ctxf[0:64, o:o + 512],
                                in0=craw[0:64, :], in1=Dt[:, :], op=MULT)
                        else:
                            ctxd = divp.tile([64, 512], BF16, tag="ctxd")
                            nc.vector.tensor_tensor(
                                out=ctxd[:, :], in0=craw[0:64, :],
                                in1=Dt[:, :], op=MULT)
                            nc.gpsimd.dma_start(ctxf[64:128, o:o + 512],
                                                ctxd[:, :])

                    # out-projection + store for this q block
                    for tt in range(4):
                        t = 4 * j + tt
                        ot = ostg.tile([128, 1024], BF16, tag="ot")
                        for oc in range(2):
                            op = ops.tile([128, 512], F32, tag="op")
                            for ci in range(2):
                                nc.tensor.matmul(
                                    op[:, :],
                                    ctxf[:, ci * S + t * 128:ci * S + t * 128 + 128],
                                    wo_sb[:, ci * HID + oc * 512:
                                          ci * HID + oc * 512 + 512],
                                    start=(ci == 0), stop=(ci == 1))
                            nc.vector.tensor_copy(
                                ot[:, oc * 512:(oc + 1) * 512], op[:, :])
                        eng = nc.sync if tt % 2 == 0 else nc.gpsimd
                        eng.dma_start(out[t * 128:(t + 1) * 128, :], ot[:, :])

    nc.compile()
    return nc


_NC = None


def _get_nc():
    global _NC
    if _NC is None:
        _NC = build_nc()
    return _NC


def make_in_maps(x, Wq, bq, Wk, bk, Wv, bv, Wo, bo):
    bf16 = ml_dtypes.bfloat16
    x = np.asarray(x, np.float32)
    in_maps = []
    for core in range(NCORES):
        b, g = core // 4, core % 4
        sl = slice(g * QC, (g + 1) * QC)
        in_maps.append({
            "xT": np.ascontiguousarray(x[b].T).astype(np.float16),
            "wq": (np.ascontiguousarray(np.asarray(Wq, np.float32)[:, sl])
                   * 0.125).astype(np.float16),
            "wk": np.ascontiguousarray(
                np.asarray(Wk, np.float32)[:, sl]).astype(np.float16),
            "wv": np.ascontiguousarray(
                np.asarray(Wv, np.float32)[:, sl]).astype(np.float16),
            "wo": np.ascontiguousarray(
                np.asarray(Wo, np.float32)[sl, :]).astype(bf16),
            "bq": (np.asarray(bq, np.float32)[sl] * 0.125).astype(np.float32),
            "bk": np.asarray(bk, np.float32)[sl].astype(np.float32),
            "ident": np.eye(128, dtype=np.float32).astype(bf16),
        })
    return in_maps


def combine_outputs(core_outs, Wv_bias_term):
    full = np.empty((B, S, HID), np.float32)
    for b in range(B):
        acc = core_outs[4 * b].astype(np.float32).copy()
        for g in range(1, 4):
            acc += core_outs[4 * b + g]
        full[b] = acc + Wv_bias_term
    return full


def kernel(**inputs):
    x = np.asarray(inputs["x"], np.float32)
    Wq = np.asarray(inputs["Wq"], np.float32)
    bq = np.asarray(inputs["bq"], np.float32)
    Wk = np.asarray(inputs["Wk"], np.float32)
    bk = np.asarray(inputs["bk"], np.float32)
    Wv = np.asarray(inputs["Wv"], np.float32)
    bv = np.asarray(inputs["bv"], np.float32)
    Wo = np.asarray(inputs["Wo"], np.float32)
    bo = np.asarray(inputs["bo"], np.float32)

    nc = _get_nc()
    in_maps = make_in_maps(x, Wq, bq, Wk, bk, Wv, bv, Wo, bo)
    res = run_bass_kernel_spmd(nc, in_maps, core_ids=list(range(NCORES)))
    core_outs = [res.results[c]["out"] for c in range(NCORES)]
    bias_term = (bv @ Wo + bo).astype(np.float32)
    return combine_outputs(core_outs, bias_term)


# revision 9
# speedup vs baseline: 1.0498x; 1.0498x over previous
"""Multi-head attention (B=2, S=2048, H=1024, 16 heads) on 8 TRN2 NeuronCores.

Sharding (tensor-parallel heads x data-parallel batch, per the hint):
  core c -> batch b = c // 4, head group g = c % 4 (4 heads each).
Each core computes Q^T/K^T (fp16, duplicated across partition halves),
V (bf16, natural layout with a ones column), per-head scores^T with the
two tok_k chunks of a pair on distinct PE row groups (explicit
tile_position -> concurrent row tiles), probs = exp(scores) on the Scalar
engine exclusively, ctx via an M=65 ones-augmented V stationary (softmax
denominator rides in row 64), division on DVE/GPSIMD, and the partial
out-projection.  The 4 partial outputs per batch are summed on the host
during unsharding (Megatron-style TP partial sums); bv/bo fold into a
host-side additive constant (bv @ Wo + bo), exactly.

v3b: attention is q-block-major (j outer, heads inner, batched per (h,j):
8x(scores pair + exp) then 16 ctx matmuls then the division) so the
out-projection and output DMA for block j overlap block j+1's attention
instead of serializing in a ~50us tail.  Division chains are buffered
(bufs=4) to overlap across heads.  fp8 was evaluated and rejected:
softmax concentration exposes the ~4% elementwise quantization error
nearly unattenuated under the max-error metric (3.8e-2 > 2e-2).
"""

import ml_dtypes
import numpy as np

import concourse.bacc as bacc
import concourse.mybir as mybir
import concourse.tile as tile
from concourse.bass_utils import run_bass_kernel_spmd

NCORES = 8
B, S, HID = 2, 2048, 1024
NH, HD = 16, 64
HPC = 4            # heads per core
QC = HPC * HD      # 256 local projection cols per core
HC = HID // 128    # 8 hidden chunks
TC = S // 128      # 16 token chunks
TB = S // 512      # 4 token blocks
NJ = TB

F32 = mybir.dt.float32
BF16 = mybir.dt.bfloat16
FP16 = mybir.dt.float16
EXP = mybir.ActivationFunctionType.Exp
MULT = mybir.AluOpType.mult


def build_nc():
    nc = bacc.Bacc("TRN2", target_bir_lowering=False, debug=False,
                   num_devices=NCORES)
    xT = nc.declare_dram_parameter("xT", [HID, S], FP16, isOutput=False)
    wq = nc.declare_dram_parameter("wq", [HID, QC], FP16, isOutput=False)
    wk = nc.declare_dram_parameter("wk", [HID, QC], FP16, isOutput=False)
    wv = nc.declare_dram_parameter("wv", [HID, QC], FP16, isOutput=False)
    wo = nc.declare_dram_parameter("wo", [QC, HID], BF16, isOutput=False)
    bq = nc.declare_dram_parameter("bq", [QC], F32, isOutput=False)
    bk = nc.declare_dram_parameter("bk", [QC], F32, isOutput=False)
    ident = nc.declare_dram_parameter("ident", [128, 128], F32, isOutput=False)
    out = nc.declare_dram_parameter("out", [S, HID], BF16, isOutput=True)

    with tile.TileContext(nc) as tc:
        with (
            tc.tile_pool(name="const", bufs=1) as constp,
            tc.tile_pool(name="qkv", bufs=1) as qkvp,
        ):
            wo_sb = constp.tile([128, 2 * HID], BF16)
            bq_sb = constp.tile([128, 2], F32)
            bk_sb = constp.tile([128, 2], F32)
            idf_sb = constp.tile([128, 128], F32)
            idb_sb = constp.tile([128, 128], BF16)
            # Q^T/K^T per head, duplicated across both partition halves so
            # the chunk-pair scores matmuls run on distinct PE row groups.
            qt2 = qkvp.tile([128, HPC * S], FP16)
            kt2 = qkvp.tile([128, HPC * S], FP16)
            # Natural V (bf16) with a ones column at col 64 of each 128-wide
            # per-head strip: the ctx matmul's M=65 stationary computes ctx
            # rows 0..63 plus the softmax denominator in row 64.
            v_sb = qkvp.tile([128, TC * HPC * 128], BF16)
            vt_sb = qkvp.tile([128, 2 * S], BF16)
            ctxf_sb = qkvp.tile([128, 2 * S], BF16)

            nc.vector.memset(
                v_sb[:, :].rearrange("p (s e) -> p s e", e=128)[:, :, HD:HD + 1],
                1.0)

            # ---- phase 1: projections (baseline-proven structure) ---------
            with (
                tc.tile_pool(name="xw", bufs=1) as xwp,
                tc.tile_pool(name="ps1", bufs=2, space="PSUM") as ps1,
            ):
                xT_sb = xwp.tile([128, HC * S], FP16)
                wq_sb = xwp.tile([128, HC * QC], FP16)
                wk_sb = xwp.tile([128, HC * QC], FP16)
                wv_sb = xwp.tile([128, HC * QC], FP16)
                # wv + the first two xT chunks get the DMA engines to
                # themselves; later inputs are paced behind early V^T
                # matmuls so the first compute isn't stuck behind the whole
                # 6 MB input load.
                xt_dmas = {}
                for hc in range(HC):
                    r = slice(hc * 128, (hc + 1) * 128)
                    nc.scalar.dma_start(wv_sb[:, hc * QC:(hc + 1) * QC],
                                        wv[r, :])
                    eng = nc.sync if hc % 2 == 0 else nc.scalar
                    if hc == 0:
                        for jb in range(TB):
                            xt_dmas[hc] = eng.dma_start(
                                xT_sb[:, hc * S + jb * 512:hc * S + (jb + 1) * 512],
                                xT[r, jb * 512:(jb + 1) * 512])
                    else:
                        xt_dmas[hc] = eng.dma_start(
                            xT_sb[:, hc * S:(hc + 1) * S], xT[r, :])
                nc.scalar.dma_start(idf_sb[:, :], ident[:, :])
                nc.vector.tensor_copy(idb_sb[:, :], idf_sb[:, :])
                for ci in range(2):
                    nc.sync.dma_start(bq_sb[:, ci:ci + 1],
                                      bq[ci * 128:(ci + 1) * 128])
                    nc.sync.dma_start(bk_sb[:, ci:ci + 1],
                                      bk[ci * 128:(ci + 1) * 128])
                qk_dmas = []
                for hc in range(HC):
                    r = slice(hc * 128, (hc + 1) * 128)
                    qk_dmas.append(nc.sync.dma_start(
                        wq_sb[:, hc * QC:(hc + 1) * QC], wq[r, :]))
                    qk_dmas.append(nc.sync.dma_start(
                        wk_sb[:, hc * QC:(hc + 1) * QC], wk[r, :]))

                # V^T first (kept in SBUF; transposed on the PE below)
                vt_mms = {}
                for ci in range(2):
                    ps = ps1.tile([128, S], F32, tag="ps1")
                    for hc in range(HC):
                        for jb in range(TB):
                            mm = nc.tensor.matmul(
                                ps[:, jb * 512:(jb + 1) * 512],
                                wv_sb[:, hc * QC + ci * 128:
                                      hc * QC + ci * 128 + 128],
                                xT_sb[:, hc * S + jb * 512:
                                      hc * S + jb * 512 + 512],
                                start=(hc == 0), stop=(hc == HC - 1))
                            vt_mms[(ci, hc, jb)] = mm
                    nc.vector.tensor_copy(vt_sb[:, ci * S:(ci + 1) * S], ps[:])
                for hc in range(2, HC):
                    tile.add_dep_helper(xt_dmas[hc].ins, vt_mms[(0, hc - 2, 3)].ins,
                                        reason="pace xT input load")
                for i, d in enumerate(qk_dmas):
                    src_mm = vt_mms[(0, min(i // 2, HC - 1), 1)]
                    tile.add_dep_helper(d.ins, src_mm.ins, reason="pace w input load")
                for ci in range(2):
                    d = nc.scalar.dma_start(
                        wo_sb[:, ci * HID:(ci + 1) * HID],
                        wo[ci * 128:(ci + 1) * 128, :])
                    tile.add_dep_helper(d.ins, vt_mms[(1, 3 + 2 * ci, 0)].ins,
                                        reason="pace wo load")

                # Q^T and K^T, written into the duplicated per-head layout
                for ci in range(2):
                    for w_sb, b_sb, dst in ((wq_sb, bq_sb, qt2),
                                            (wk_sb, bk_sb, kt2)):
                        ps = ps1.tile([128, S], F32, tag="ps1")
                        for hc in range(HC):
                            for jb in range(TB):
                                nc.tensor.matmul(
                                    ps[:, jb * 512:(jb + 1) * 512],
                                    w_sb[:, hc * QC + ci * 128:
                                         hc * QC + ci * 128 + 128],
                                    xT_sb[:, hc * S + jb * 512:
                                          hc * S + jb * 512 + 512],
                                    start=(hc == 0), stop=(hc == HC - 1))
                        hA, hB = 2 * ci, 2 * ci + 1
                        nc.vector.tensor_scalar_add(
                            dst[0:64, hA * S:(hA + 1) * S], ps[0:64, :],
                            b_sb[0:64, ci:ci + 1])
                        nc.vector.tensor_scalar_add(
                            dst[64:128, hB * S:(hB + 1) * S], ps[64:128, :],
                            b_sb[64:128, ci:ci + 1])
                        nc.sync.dma_start(dst[64:128, hA * S:(hA + 1) * S],
                                          dst[0:64, hA * S:(hA + 1) * S])
                        nc.scalar.dma_start(dst[0:64, hB * S:(hB + 1) * S],
                                            dst[64:128, hB * S:(hB + 1) * S])

            # V^T -> V via PE transpose-mode, then a strided DVE copy into
            # the ones-padded layout.
            with tc.tile_pool(name="trp", bufs=4, space="PSUM") as trp:
                for ci in range(2):
                    for t in range(TC):
                        tp = trp.tile([128, 128], BF16, tag="tr")
                        nc.tensor.transpose(
                            tp[:, :],
                            vt_sb[:, ci * S + t * 128:ci * S + t * 128 + 128],
                            idb_sb[:, :])
                        dst = v_sb[:, (t * HPC + 2 * ci) * 128:
                                   (t * HPC + 2 * ci + 2) * 128].rearrange(
                            "p (h e) -> p h e", h=2)[:, :, 0:HD]
                        srcv = tp[:, :].rearrange("p (h e) -> p h e", h=2)[:, :, 0:HD]
                        nc.vector.tensor_copy(dst, srcv)

            # ---- phase 2: attention, q-block-major ------------------------
            with (
                tc.tile_pool(name="probs", bufs=6) as probsp,
                tc.tile_pool(name="div", bufs=4) as divp,
                tc.tile_pool(name="ostg", bufs=3) as ostg,
                tc.tile_pool(name="scps", bufs=2, space="PSUM") as scps,
                tc.tile_pool(name="ctps", bufs=2, space="PSUM") as ctps,
                tc.tile_pool(name="ops", bufs=2, space="PSUM") as ops,
            ):
                for j in range(NJ):
                    for h in range(HPC):
                        hS = h * S
                        ctx_ps = ctps.tile([128, 512], F32, tag="ctx")
                        probs_tiles = []
                        for cp in range(TC // 2):
                            c0, c1 = 2 * cp, 2 * cp + 1
                            sp = scps.tile([128, 1024], F32, tag="sc")
                            nc.tensor.matmul(
                                sp[:, 0:512],
                                kt2[0:64, hS + c0 * 128:hS + c0 * 128 + 128],
                                qt2[0:64, hS + j * 512:hS + j * 512 + 512],
                                start=True, stop=True, tile_position=(0, 0))
                            nc.tensor.matmul(
                                sp[:, 512:1024],
                                kt2[64:128, hS + c1 * 128:hS + c1 * 128 + 128],
                                qt2[64:128, hS + j * 512:hS + j * 512 + 512],
                                start=True, stop=True, tile_position=(64, 0))
                            probs = probsp.tile([128, 1024], BF16, tag="probs")
                            nc.scalar.activation(probs[:, :], sp[:, :], EXP)
                            probs_tiles.append(probs)
                        for cp in range(TC // 2):
                            c0, c1 = 2 * cp, 2 * cp + 1
                            probs = probs_tiles[cp]
                            for ck, coff in ((c0, 0), (c1, 512)):
                                strip = (ck * HPC + h) * 128
                                nc.tensor.matmul(
                                    ctx_ps[0:HD + 1, :],
                                    v_sb[:, strip:strip + HD + 1],
                                    probs[:, coff:coff + 512],
                                    start=(cp == 0 and ck == c0),
                                    stop=(cp == TC // 2 - 1 and ck == c1))
                        # divide by the softmax denominator (row 64)
                        ci, lo = h // 2, (h % 2) * 64
                        craw = divp.tile([65, 512], F32, tag="craw")
                        nc.vector.tensor_copy(craw[0:65, :], ctx_ps[0:65, :])
                        denr = divp.tile([128, 4], F32, tag="denr")
                        nc.sync.dma_start(denr[:, :], craw[64:65, :])
                        recr = divp.tile([128, 4], F32, tag="recr")
                        nc.vector.reciprocal(recr[:, :], denr[:, :])
                        rrow = divp.tile([1, 512], F32, tag="rrow")
                        nc.sync.dma_start(rrow[:, :], recr[:, :])
                        Dt = divp.tile([64, 512], F32, tag="Dt")
                        nc.gpsimd.partition_broadcast(Dt[:, :], rrow[0:1, :])
                        o = ci * S + j * 512
                        if lo == 0:
                            nc.vector.tensor_tensor(
                                out=ctxf_sb[0:64, o:o + 512],
                                in0=craw[0:64, :], in1=Dt[:, :], op=MULT)
                        else:
                            ctxd = divp.tile([64, 512], BF16, tag="ctxd")
                            nc.vector.tensor_tensor(
                                out=ctxd[:, :], in0=craw[0:64, :],
                                in1=Dt[:, :], op=MULT)
                            nc.gpsimd.dma_start(ctxf_sb[64:128, o:o + 512],
                                                ctxd[:, :])

                    # out-projection + store for this q block
                    for tt in range(4):
                        t = 4 * j + tt
                        ot = ostg.tile([128, 1024], BF16, tag="ot")
                        for oc in range(2):
                            op = ops.tile([128, 512], F32, tag="op")
                            for ci in range(2):
                                nc.tensor.matmul(
                                    op[:, :],
                                    ctxf_sb[:, ci * S + t * 128:ci * S + t * 128 + 128],
                                    wo_sb[:, ci * HID + oc * 512:
                                          ci * HID + oc * 512 + 512],
                                    start=(ci == 0), stop=(ci == 1))
                            nc.vector.tensor_copy(
                                ot[:, oc * 512:(oc + 1) * 512], op[:, :])
                        eng = nc.sync if tt % 2 == 0 else nc.gpsimd
                        eng.dma_start(out[t * 128:(t + 1) * 128, :], ot[:, :])

    nc.compile()
    return nc


_NC = None


def _get_nc():
    global _NC
    if _NC is None:
        _NC = build_nc()
    return _NC


def make_in_maps(x, Wq, bq, Wk, bk, Wv, bv, Wo, bo):
    bf16 = ml_dtypes.bfloat16
    x = np.asarray(x, np.float32)
    in_maps = []
    for core in range(NCORES):
        b, g = core // 4, core % 4
        sl = slice(g * QC, (g + 1) * QC)
        in_maps.append({
            "xT": np.ascontiguousarray(x[b].T).astype(np.float16),
            "wq": (np.ascontiguousarray(np.asarray(Wq, np.float32)[:, sl])
                   * 0.125).astype(np.float16),
            "wk": np.ascontiguousarray(
                np.asarray(Wk, np.float32)[:, sl]).astype(np.float16),
            "wv": np.ascontiguousarray(
                np.asarray(Wv, np.float32)[:, sl]).astype(np.float16),
            "wo": np.ascontiguousarray(
                np.asarray(Wo, np.float32)[sl, :]).astype(bf16),
            "bq": (np.asarray(bq, np.float32)[sl] * 0.125).astype(np.float32),
            "bk": np.asarray(bk, np.float32)[sl].astype(np.float32),
            "ident": np.eye(128, dtype=np.float32),
        })
    return in_maps


def combine_outputs(core_outs, Wv_bias_term):
    full = np.empty((B, S, HID), np.float32)
    for b in range(B):
        acc = core_outs[4 * b].astype(np.float32).copy()
        for g in range(1, 4):
            acc += core_outs[4 * b + g]
        full[b] = acc + Wv_bias_term
    return full


def kernel(**inputs):
    x = np.asarray(inputs["x"], np.float32)
    Wq = np.asarray(inputs["Wq"], np.float32)
    bq = np.asarray(inputs["bq"], np.float32)
    Wk = np.asarray(inputs["Wk"], np.float32)
    bk = np.asarray(inputs["bk"], np.float32)
    Wv = np.asarray(inputs["Wv"], np.float32)
    bv = np.asarray(inputs["bv"], np.float32)
    Wo = np.asarray(inputs["Wo"], np.float32)
    bo = np.asarray(inputs["bo"], np.float32)

    nc = _get_nc()
    in_maps = make_in_maps(x, Wq, bq, Wk, bk, Wv, bv, Wo, bo)
    res = run_bass_kernel_spmd(nc, in_maps, core_ids=list(range(NCORES)))
    core_outs = [res.results[c]["out"] for c in range(NCORES)]
    bias_term = (bv @ Wo + bo).astype(np.float32)
    return combine_outputs(core_outs, bias_term)


# revision 13
# speedup vs baseline: 1.2349x; 1.1763x over previous
"""Multi-head attention (B=2, S=2048, H=1024, 16 heads) on 8 TRN2 NeuronCores.

Sharding (tensor-parallel heads x data-parallel batch, per the hint):
  core c -> batch b = c // 4, head group g = c % 4 (4 heads each).
Each core computes Q^T/K^T (fp16, duplicated across partition halves),
V (bf16, natural layout with a ones column), per-head scores^T with the
two tok_k chunks of a pair on distinct PE row groups (explicit
tile_position -> concurrent row tiles), probs = exp(scores) on the Scalar
engine exclusively, ctx via an M=65 ones-augmented V stationary (softmax
denominator rides in row 64), division on DVE/GPSIMD, and the partial
out-projection.  The 4 partial outputs per batch are summed on the host
during unsharding (Megatron-style TP partial sums); bv/bo fold into a
host-side additive constant (bv @ Wo + bo), exactly.

v3b: attention is q-block-major (j outer, heads inner, batched per (h,j):
8x(scores pair + exp) then 16 ctx matmuls then the division) so the
out-projection and output DMA for block j overlap block j+1's attention
instead of serializing in a ~50us tail.  Division chains are buffered
(bufs=4) to overlap across heads.  fp8 was evaluated and rejected:
softmax concentration exposes the ~4% elementwise quantization error
nearly unattenuated under the max-error metric (3.8e-2 > 2e-2).
"""

import ml_dtypes
import numpy as np

import concourse.bacc as bacc
import concourse.mybir as mybir
import concourse.tile as tile
from concourse.bass_utils import run_bass_kernel_spmd

NCORES = 8
B, S, HID = 2, 2048, 1024
NH, HD = 16, 64
HPC = 4            # heads per core
QC = HPC * HD      # 256 local projection cols per core
HC = HID // 128    # 8 hidden chunks
TC = S // 128      # 16 token chunks
TB = S // 512      # 4 token blocks
NJ = TB

F32 = mybir.dt.float32
BF16 = mybir.dt.bfloat16
FP16 = mybir.dt.float16
EXP = mybir.ActivationFunctionType.Exp
MULT = mybir.AluOpType.mult


def build_nc():
    nc = bacc.Bacc("TRN2", target_bir_lowering=False, debug=False,
                   num_devices=NCORES)
    xT = nc.declare_dram_parameter("xT", [HID, S], FP16, isOutput=False)
    wq = nc.declare_dram_parameter("wq", [HID, QC], FP16, isOutput=False)
    wk = nc.declare_dram_parameter("wk", [HID, QC], FP16, isOutput=False)
    wv = nc.declare_dram_parameter("wv", [HID, QC], FP16, isOutput=False)
    wo = nc.declare_dram_parameter("wo", [QC, HID], BF16, isOutput=False)
    bq = nc.declare_dram_parameter("bq", [QC], F32, isOutput=False)
    bk = nc.declare_dram_parameter("bk", [QC], F32, isOutput=False)
    ident = nc.declare_dram_parameter("ident", [128, 128], F32, isOutput=False)
    out = nc.declare_dram_parameter("out", [S, HID], BF16, isOutput=True)

    with tile.TileContext(nc) as tc:
        with (
            tc.tile_pool(name="const", bufs=1) as constp,
            tc.tile_pool(name="qkv", bufs=1) as qkvp,
        ):
            wo_sb = constp.tile([128, 2 * HID], BF16)
            bq_sb = constp.tile([128, 2], F32)
            bk_sb = constp.tile([128, 2], F32)
            idf_sb = constp.tile([128, 128], F32)
            idb_sb = constp.tile([128, 128], BF16)
            # Q^T/K^T per head, duplicated across both partition halves so
            # the chunk-pair scores matmuls run on distinct PE row groups.
            qt2 = qkvp.tile([128, HPC * S], FP16)
            kt2 = qkvp.tile([128, HPC * S], FP16)
            # Natural V (bf16) with a ones column at col 64 of each 128-wide
            # per-head strip: the ctx matmul's M=65 stationary computes ctx
            # rows 0..63 plus the softmax denominator in row 64.
            v_sb = qkvp.tile([128, TC * HPC * 128], BF16)
            vt_sb = qkvp.tile([128, 2 * S], BF16)
            ctxf_sb = qkvp.tile([128, 2 * S], BF16)

            nc.vector.memset(
                v_sb[:, :].rearrange("p (s e) -> p s e", e=128)[:, :, HD:HD + 1],
                1.0)

            # ---- phase 1: projections (baseline-proven structure) ---------
            with (
                tc.tile_pool(name="xw", bufs=1) as xwp,
                tc.tile_pool(name="ps1", bufs=2, space="PSUM") as ps1,
            ):
                xT_sb = xwp.tile([128, HC * S], FP16)
                wq_sb = xwp.tile([128, HC * QC], FP16)
                wk_sb = xwp.tile([128, HC * QC], FP16)
                wv_sb = xwp.tile([128, HC * QC], FP16)
                # wv + the first two xT chunks get the DMA engines to
                # themselves; later inputs are paced behind early V^T
                # matmuls so the first compute isn't stuck behind the whole
                # 6 MB input load.
                xt_dmas = {}
                for hc in range(HC):
                    r = slice(hc * 128, (hc + 1) * 128)
                    nc.scalar.dma_start(wv_sb[:, hc * QC:(hc + 1) * QC],
                                        wv[r, :])
                    eng = nc.sync if hc % 2 == 0 else nc.scalar
                    if hc == 0:
                        for jb in range(TB):
                            xt_dmas[hc] = eng.dma_start(
                                xT_sb[:, hc * S + jb * 512:hc * S + (jb + 1) * 512],
                                xT[r, jb * 512:(jb + 1) * 512])
                    else:
                        xt_dmas[hc] = eng.dma_start(
                            xT_sb[:, hc * S:(hc + 1) * S], xT[r, :])
                nc.scalar.dma_start(idf_sb[:, :], ident[:, :])
                nc.vector.tensor_copy(idb_sb[:, :], idf_sb[:, :])
                for ci in range(2):
                    nc.sync.dma_start(bq_sb[:, ci:ci + 1],
                                      bq[ci * 128:(ci + 1) * 128])
                    nc.sync.dma_start(bk_sb[:, ci:ci + 1],
                                      bk[ci * 128:(ci + 1) * 128])
                qk_dmas = []
                for hc in range(HC):
                    r = slice(hc * 128, (hc + 1) * 128)
                    qk_dmas.append(nc.sync.dma_start(
                        wq_sb[:, hc * QC:(hc + 1) * QC], wq[r, :]))
                    qk_dmas.append(nc.sync.dma_start(
                        wk_sb[:, hc * QC:(hc + 1) * QC], wk[r, :]))

                # V^T first (kept in SBUF; transposed on the PE below)
                vt_mms = {}
                for ci in range(2):
                    ps = ps1.tile([128, S], F32, tag="ps1")
                    for hc in range(HC):
                        for jb in range(TB):
                            mm = nc.tensor.matmul(
                                ps[:, jb * 512:(jb + 1) * 512],
                                wv_sb[:, hc * QC + ci * 128:
                                      hc * QC + ci * 128 + 128],
                                xT_sb[:, hc * S + jb * 512:
                                      hc * S + jb * 512 + 512],
                                start=(hc == 0), stop=(hc == HC - 1))
                            vt_mms[(ci, hc, jb)] = mm
                    nc.vector.tensor_copy(vt_sb[:, ci * S:(ci + 1) * S], ps[:])
                for hc in range(2, HC):
                    tile.add_dep_helper(xt_dmas[hc].ins, vt_mms[(0, hc - 2, 3)].ins,
                                        reason="pace xT input load")
                for i, d in enumerate(qk_dmas):
                    src_mm = vt_mms[(0, min(i // 2, HC - 1), 1)]
                    tile.add_dep_helper(d.ins, src_mm.ins, reason="pace w input load")
                for ci in range(2):
                    d = nc.scalar.dma_start(
                        wo_sb[:, ci * HID:(ci + 1) * HID],
                        wo[ci * 128:(ci + 1) * 128, :])
                    tile.add_dep_helper(d.ins, vt_mms[(1, 3 + 2 * ci, 0)].ins,
                                        reason="pace wo load")

                # Q^T and K^T, written into the duplicated per-head layout
                for ci in range(2):
                    for w_sb, b_sb, dst in ((wq_sb, bq_sb, qt2),
                                            (wk_sb, bk_sb, kt2)):
                        ps = ps1.tile([128, S], F32, tag="ps1")
                        for hc in range(HC):
                            for jb in range(TB):
                                nc.tensor.matmul(
                                    ps[:, jb * 512:(jb + 1) * 512],
                                    w_sb[:, hc * QC + ci * 128:
                                         hc * QC + ci * 128 + 128],
                                    xT_sb[:, hc * S + jb * 512:
                                          hc * S + jb * 512 + 512],
                                    start=(hc == 0), stop=(hc == HC - 1))
                        hA, hB = 2 * ci, 2 * ci + 1
                        nc.vector.tensor_scalar_add(
                            dst[0:64, hA * S:(hA + 1) * S], ps[0:64, :],
                            b_sb[0:64, ci:ci + 1])
                        nc.vector.tensor_scalar_add(
                            dst[64:128, hB * S:(hB + 1) * S], ps[64:128, :],
                            b_sb[64:128, ci:ci + 1])
                        nc.sync.dma_start(dst[64:128, hA * S:(hA + 1) * S],
                                          dst[0:64, hA * S:(hA + 1) * S])
                        nc.scalar.dma_start(dst[0:64, hB * S:(hB + 1) * S],
                                            dst[64:128, hB * S:(hB + 1) * S])

            # V^T -> V via PE transpose-mode, then a strided DVE copy into
            # the ones-padded layout.
            with tc.tile_pool(name="trp", bufs=4, space="PSUM") as trp:
                for ci in range(2):
                    for t in range(TC):
                        tp = trp.tile([128, 128], BF16, tag="tr")
                        nc.tensor.transpose(
                            tp[:, :],
                            vt_sb[:, ci * S + t * 128:ci * S + t * 128 + 128],
                            idb_sb[:, :])
                        dst = v_sb[:, (t * HPC + 2 * ci) * 128:
                                   (t * HPC + 2 * ci + 2) * 128].rearrange(
                            "p (h e) -> p h e", h=2)[:, :, 0:HD]
                        srcv = tp[:, :].rearrange("p (h e) -> p h e", h=2)[:, :, 0:HD]
                        nc.vector.tensor_copy(dst, srcv)

            # ---- phase 2: attention, q-block-major ------------------------
            # Units (j, h) are software-pipelined with LEAD=1: scores+exp of
            # unit k+1 are emitted before ctx of unit k so the PE's ctx block
            # never starves the Scalar engine's exp stream.  ctx accumulates
            # in two alternating PSUM banks (A: even chunks, B: odd chunks)
            # to avoid same-bank drain contention; the division fuses A+B.
            with (
                tc.tile_pool(name="probs", bufs=4) as probsp,
                tc.tile_pool(name="div", bufs=4) as divp,
                tc.tile_pool(name="ostg", bufs=3) as ostg,
                tc.tile_pool(name="scps", bufs=2, space="PSUM") as scps,
                tc.tile_pool(name="ctps", bufs=1, space="PSUM") as ctps,
                tc.tile_pool(name="ops", bufs=2, space="PSUM") as ops,
            ):
                units = [(j, h) for j in range(NJ) for h in range(HPC)]
                probs_map = {}
                ctx_map = {}

                def emit_scores(j, h):
                    hS = h * S
                    # two 4096-wide probs tiles per unit; each holds 4 exps
                    probs_map[(j, h)] = probsp.tile(
                        [128, 4 * 1024], BF16, tag="probs",
                        name=f"probs_a_{j}_{h}")
                    for cp in range(TC // 2):
                        if cp == 4:
                            probs_map[(j, h, "b")] = probsp.tile(
                                [128, 4 * 1024], BF16, tag="probs",
                                name=f"probs_b_{j}_{h}")
                        c0, c1 = 2 * cp, 2 * cp + 1
                        sp = scps.tile([128, 1024], F32, tag="sc")
                        nc.tensor.matmul(
                            sp[:, 0:512],
                            kt2[0:64, hS + c0 * 128:hS + c0 * 128 + 128],
                            qt2[0:64, hS + j * 512:hS + j * 512 + 512],
                            start=True, stop=True, tile_position=(0, 0))
                        nc.tensor.matmul(
                            sp[:, 512:1024],
                            kt2[64:128, hS + c1 * 128:hS + c1 * 128 + 128],
                            qt2[64:128, hS + j * 512:hS + j * 512 + 512],
                            start=True, stop=True, tile_position=(64, 0))
                        pt = probs_map[(j, h)] if cp < 4 else probs_map[(j, h, "b")]
                        o = (cp % 4) * 1024
                        nc.scalar.activation(pt[:, o:o + 1024], sp[:, :], EXP)

                def emit_ctx(j, h):
                    ctxA = ctps.tile([128, 512], F32, tag="ctxA")
                    ctxB = ctps.tile([128, 512], F32, tag="ctxB")
                    ctx_map[(j, h)] = (ctxA, ctxB)
                    pa = probs_map.pop((j, h))
                    pb = probs_map.pop((j, h, "b"))
                    for cp in range(TC // 2):
                        c0, c1 = 2 * cp, 2 * cp + 1
                        probs = pa if cp < 4 else pb
                        o = (cp % 4) * 1024
                        for ck, coff, cps in ((c0, 0, ctxA), (c1, 512, ctxB)):
                            strip = (ck * HPC + h) * 128
                            nc.tensor.matmul(
                                cps[0:HD + 1, :],
                                v_sb[:, strip:strip + HD + 1],
                                probs[:, o + coff:o + coff + 512],
                                start=(cp == 0), stop=(cp == TC // 2 - 1))

                def emit_division(j, h):
                    ci, lo = h // 2, (h % 2) * 64
                    ctxA, ctxB = ctx_map.pop((j, h))
                    cA = divp.tile([65, 512], F32, tag="cA")
                    nc.vector.tensor_copy(cA[0:65, :], ctxA[0:65, :])
                    craw = divp.tile([65, 512], F32, tag="craw")
                    nc.vector.tensor_tensor(out=craw[0:65, :],
                                            in0=cA[0:65, :],
                                            in1=ctxB[0:65, :],
                                            op=mybir.AluOpType.add)
                    denr = divp.tile([128, 4], F32, tag="denr")
                    nc.sync.dma_start(denr[:, :], craw[64:65, :])
                    recr = divp.tile([128, 4], F32, tag="recr")
                    nc.vector.reciprocal(recr[:, :], denr[:, :])
                    rrow = divp.tile([1, 512], F32, tag="rrow")
                    nc.sync.dma_start(rrow[:, :], recr[:, :])
                    Dt = divp.tile([64, 512], F32, tag="Dt")
                    nc.gpsimd.partition_broadcast(Dt[:, :], rrow[0:1, :])
                    o = ci * S + j * 512
                    if lo == 0:
                        nc.vector.tensor_tensor(
                            out=ctxf_sb[0:64, o:o + 512],
                            in0=craw[0:64, :], in1=Dt[:, :], op=MULT)
                    else:
                        ctxd = divp.tile([64, 512], BF16, tag="ctxd")
                        nc.vector.tensor_tensor(
                            out=ctxd[:, :], in0=craw[0:64, :],
                            in1=Dt[:, :], op=MULT)
                        nc.gpsimd.dma_start(ctxf_sb[64:128, o:o + 512],
                                            ctxd[:, :])

                def emit_outproj(j):
                    for tt in range(4):
                        t = 4 * j + tt
                        ot = ostg.tile([128, 1024], BF16, tag="ot")
                        for oc in range(2):
                            op = ops.tile([128, 512], F32, tag="op")
                            for ci in range(2):
                                nc.tensor.matmul(
                                    op[:, :],
                                    ctxf_sb[:, ci * S + t * 128:ci * S + t * 128 + 128],
                                    wo_sb[:, ci * HID + oc * 512:
                                          ci * HID + oc * 512 + 512],
                                    start=(ci == 0), stop=(ci == 1))
                            nc.vector.tensor_copy(
                                ot[:, oc * 512:(oc + 1) * 512], op[:, :])
                        eng = nc.sync if tt % 2 == 0 else nc.gpsimd
                        eng.dma_start(out[t * 128:(t + 1) * 128, :], ot[:, :])

                LEAD = 1
                for k in range(len(units) + LEAD):
                    if k < len(units):
                        emit_scores(*units[k])
                    if k >= LEAD:
                        j, h = units[k - LEAD]
                        emit_ctx(j, h)
                        emit_division(j, h)
                        if h == HPC - 1:
                            emit_outproj(j)

    nc.compile()
    return nc


_NC = None


def _get_nc():
    global _NC
    if _NC is None:
        _NC = build_nc()
    return _NC


def make_in_maps(x, Wq, bq, Wk, bk, Wv, bv, Wo, bo):
    bf16 = ml_dtypes.bfloat16
    x = np.asarray(x, np.float32)
    in_maps = []
    for core in range(NCORES):
        b, g = core // 4, core % 4
        sl = slice(g * QC, (g + 1) * QC)
        in_maps.append({
            "xT": np.ascontiguousarray(x[b].T).astype(np.float16),
            "wq": (np.ascontiguousarray(np.asarray(Wq, np.float32)[:, sl])
                   * 0.125).astype(np.float16),
            "wk": np.ascontiguousarray(
                np.asarray(Wk, np.float32)[:, sl]).astype(np.float16),
            "wv": np.ascontiguousarray(
                np.asarray(Wv, np.float32)[:, sl]).astype(np.float16),
            "wo": np.ascontiguousarray(
                np.asarray(Wo, np.float32)[sl, :]).astype(bf16),
            "bq": (np.asarray(bq, np.float32)[sl] * 0.125).astype(np.float32),
            "bk": np.asarray(bk, np.float32)[sl].astype(np.float32),
            "ident": np.eye(128, dtype=np.float32),
        })
    return in_maps


def combine_outputs(core_outs, Wv_bias_term):
    full = np.empty((B, S, HID), np.float32)
    for b in range(B):
        acc = core_outs[4 * b].astype(np.float32).copy()
        for g in range(1, 4):
            acc += core_outs[4 * b + g]
        full[b] = acc + Wv_bias_term
    return full


def kernel(**inputs):
    x = np.asarray(inputs["x"], np.float32)
    Wq = np.asarray(inputs["Wq"], np.float32)
    bq = np.asarray(inputs["bq"], np.float32)
    Wk = np.asarray(inputs["Wk"], np.float32)
    bk = np.asarray(inputs["bk"], np.float32)
    Wv = np.asarray(inputs["Wv"], np.float32)
    bv = np.asarray(inputs["bv"], np.float32)
    Wo = np.asarray(inputs["Wo"], np.float32)
    bo = np.asarray(inputs["bo"], np.float32)

    nc = _get_nc()
    in_maps = make_in_maps(x, Wq, bq, Wk, bk, Wv, bv, Wo, bo)
    res = run_bass_kernel_spmd(nc, in_maps, core_ids=list(range(NCORES)))
    core_outs = [res.results[c]["out"] for c in range(NCORES)]
    bias_term = (bv @ Wo + bo).astype(np.float32)
    return combine_outputs(core_outs, bias_term)
